# revision 4
# baseline (speedup 1.0000x reference)
"""MultiHeadAttention Trainium2 kernel (8-core SPMD).

Problem: B=2, S=4096, HID=512, NH=8 heads of HD=64.
Outputs: (output [B,S,HID] f32, attention [B,NH,S,S] f32).

Sharding: 16 (batch, head) blocks; each of 8 cores owns one batch b and 2
heads. Per core:
  - project q[b],k[b],v[b] with the weight columns of its 2 heads (bf16 PE)
  - scores block [128q, 4096k] on PE; exp on ACT (PSUM->SBUF bf16) with
    accum_out emitting softmax row sums for free
  - normalize on DVE (per-partition tensor_scalar) -> f32 attention out
  - DMA-transpose bf16 exp tiles -> PE attn@V (unnormalized), final
    projection + per-head 1/sum scaling + head-sum + bias
Host glue sums the 4 per-batch partial outputs and reassembles attention.
"""

import numpy as np
import ml_dtypes

import concourse.bass as bass
from concourse import bacc
import concourse.mybir as mybir
import concourse.tile as tile

B, S, HID, NH, HD = 2, 4096, 512, 8, 64
NCORES = 8
HPC = NH * B // NCORES  # heads per core = 2
F32 = mybir.dt.float32
BF16 = mybir.dt.float16  # 2-byte compute dtype (fp16: 10-bit mantissa)
AF = mybir.ActivationFunctionType
ALU = mybir.AluOpType


def build(s=S):
    """Build the per-core Bass program (same program on all 8 cores)."""
    nc = bacc.Bacc("TRN2", target_bir_lowering=False, debug=False,
                   num_devices=NCORES)
    NB = s // 128       # q row blocks / k tiles
    NI = HID // 128     # input-dim 128-tiles (4)
    NW = HPC * HD       # per-core projected width (128)
    # scores psum chunks: multiples of 512, at most 1536 (3 PSUM banks)
    chunks = []
    c0 = 0
    while c0 < s:
        cl = min(1536, s - c0)
        chunks.append((c0, cl))
        c0 += cl
    NCH = len(chunks)

    xq_d = nc.dram_tensor("xq", [s, HID], F32, kind="ExternalInput")
    xk_d = nc.dram_tensor("xk", [s, HID], F32, kind="ExternalInput")
    xv_d = nc.dram_tensor("xv", [s, HID], F32, kind="ExternalInput")
    wq_d = nc.dram_tensor("wq", [HID, NW], F32, kind="ExternalInput")
    wk_d = nc.dram_tensor("wk", [HID, NW], F32, kind="ExternalInput")
    wv_d = nc.dram_tensor("wv", [HID, NW], F32, kind="ExternalInput")
    wfc_d = nc.dram_tensor("wfc", [NW, HID], F32, kind="ExternalInput")
    bq_d = nc.dram_tensor("bq", [1, NW], F32, kind="ExternalInput")
    bk_d = nc.dram_tensor("bk", [1, NW], F32, kind="ExternalInput")
    bv_d = nc.dram_tensor("bv", [1, NW], F32, kind="ExternalInput")
    bfc_d = nc.dram_tensor("bfc", [1, HID], F32, kind="ExternalInput")
    madd_d = nc.dram_tensor("madd", [1, s], BF16, kind="ExternalInput")

    attn_d = nc.dram_tensor("attn", [HPC, s, s], F32, kind="ExternalOutput")
    pout_d = nc.dram_tensor("pout", [s, HID], F32, kind="ExternalOutput")

    from contextlib import ExitStack

    with tile.TileContext(nc) as tc, ExitStack() as top:
        const = top.enter_context(tc.tile_pool(name="const", bufs=1))

        # persistent tiles
        QT = [const.tile([HD + 1, s], BF16, name=f"QT{h}", tag=f"QT{h}") for h in range(HPC)]
        KT = [const.tile([HD + 1, s], BF16, name=f"KT{h}", tag=f"KT{h}") for h in range(HPC)]
        vh = [const.tile([128, NW], BF16, name=f"vh{t}", tag=f"vh{t}") for t in range(NB)]
        ctxT = [const.tile([HD, s], BF16, name=f"ctxT{h}", tag=f"ctxT{h}") for h in range(HPC)]
        wfc_sb = [const.tile([HD, HID], BF16, name=f"wfc{h}", tag=f"wfc{h}") for h in range(HPC)]
        bfc_bc = const.tile([128, HID], F32, name="bfc_bc", tag="bfc_bc")
        inv_all = const.tile([128, NB * HPC], F32, name="inv_all", tag="inv_all")
        ones_row = const.tile([1, s], BF16, name="ones_row", tag="ones_row")

        nc.vector.memset(ones_row[:], 1.0)
        for h in range(HPC):
            nc.vector.memset(QT[h][HD:HD + 1, :], 1.0)  # ones row for mask matmul
            nc.sync.dma_start(KT[h][HD:HD + 1, :], madd_d[:])  # mask additive row
        # broadcast bfc to 128 partitions (stride-0 partition AP)
        bfc_ap = bfc_d[:]
        nc.gpsimd.dma_start(
            bfc_bc[:],
            bass.AP(tensor=bfc_ap.tensor, offset=bfc_ap.offset,
                    ap=[[0, 128]] + list(bfc_ap.ap[1:])),
        )

        # ---------------- prep: weights ----------------
        with ExitStack() as prep:
            pstage = prep.enter_context(tc.tile_pool(name="pstage", bufs=3))
            pw = prep.enter_context(tc.tile_pool(name="pw", bufs=1))
            ppsum = prep.enter_context(
                tc.tile_pool(name="ppsum", bufs=4, space=bass.MemorySpace.PSUM))
            pxt = prep.enter_context(tc.tile_pool(name="pxt", bufs=1))

            w_bf = {}
            for wname, wd in (("q", wq_d), ("k", wk_d), ("v", wv_d)):
                for i in range(NI):
                    st = pstage.tile([128, NW], F32, name="wstage", tag="wstage")
                    nc.gpsimd.dma_start(st[:], wd[i * 128:(i + 1) * 128, :])
                    wb = pw.tile([128, NW], BF16, name=f"w{wname}{i}", tag=f"w{wname}{i}")
                    nc.vector.tensor_copy(wb[:], st[:])
                    w_bf[wname, i] = wb
            b_bf = {}
            for bname, bd in (("q", bq_d), ("k", bk_d), ("v", bv_d)):
                st = pstage.tile([1, NW], F32, name="bstage", tag="bstage")
                nc.gpsimd.dma_start(st[:], bd[:])
                bb = pw.tile([1, NW], BF16, name=f"b{bname}", tag=f"b{bname}")
                nc.vector.tensor_copy(bb[:], st[:])
                b_bf[bname] = bb
            # wfc rows per head -> [64, HID] bf16 tiles
            for h in range(HPC):
                st = pstage.tile([HD, HID], F32, name="fcstage", tag="fcstage")
                nc.gpsimd.dma_start(st[:], wfc_d[h * HD:(h + 1) * HD, :])
                nc.vector.tensor_copy(wfc_sb[h][:], st[:])

            # ------------ prep: transpose inputs + project ------------
            for tname, xd in (("q", xq_d), ("k", xk_d), ("v", xv_d)):
                # xT[i-tile] [128, s] bf16 via cast + DMA-transpose
                xT = [pxt.tile([128, s], BF16, name=f"xT{i}", tag=f"xT{i}") for i in range(NI)]
                for t in range(NB):
                    st = pstage.tile([128, HID], F32, name="xstage", tag="xstage")
                    nc.sync.dma_start(st[:], xd[t * 128:(t + 1) * 128, :])
                    xb = pstage.tile([128, HID], BF16, name="xcast", tag="xcast")
                    nc.vector.tensor_copy(xb[:], st[:])
                    for i in range(NI):
                        nc.scalar.dma_start_transpose(
                            xT[i][:, t * 128:(t + 1) * 128],
                            xb[:, i * 128:(i + 1) * 128])
                if tname in ("q", "k"):
                    dst = QT if tname == "q" else KT
                    for h in range(HPC):
                        for n in range(s // 512):
                            ps = ppsum.tile([HD, 512], F32, name="projqk", tag="projqk")
                            for i in range(NI):
                                nc.tensor.matmul(
                                    ps[:], w_bf[tname, i][:, h * HD:(h + 1) * HD],
                                    xT[i][:, n * 512:(n + 1) * 512],
                                    start=(i == 0), stop=False)
                            nc.tensor.matmul(
                                ps[:], b_bf[tname][:, h * HD:(h + 1) * HD],
                                ones_row[:, n * 512:(n + 1) * 512],
                                start=False, stop=True)
                            nc.scalar.copy(dst[h][:HD, n * 512:(n + 1) * 512], ps[:])
                else:
                    for t in range(NB):
                        ps = ppsum.tile([128, NW], F32, name="projv", tag="projv")
                        for i in range(NI):
                            nc.tensor.matmul(
                                ps[:], xT[i][:, t * 128:(t + 1) * 128], w_bf["v", i][:],
                                start=(i == 0), stop=False)
                        nc.tensor.matmul(
                            ps[:], ones_row[:, t * 128:(t + 1) * 128], b_bf["v"][:],
                            start=False, stop=True)
                        nc.scalar.copy(vh[t][:], ps[:])

        # ---------------- main loop ----------------
        with ExitStack() as main:
            mexp = main.enter_context(tc.tile_pool(name="mexp", bufs=3))
            mexpT = main.enter_context(tc.tile_pool(name="mexpT", bufs=2))
            mattn = main.enter_context(tc.tile_pool(name="mattn", bufs=2))
            macc = main.enter_context(tc.tile_pool(name="macc", bufs=4))
            spsum = main.enter_context(
                tc.tile_pool(name="spsum", bufs=2, space=bass.MemorySpace.PSUM))
            cpsum = main.enter_context(
                tc.tile_pool(name="cpsum", bufs=2, space=bass.MemorySpace.PSUM))

            for qb in range(NB):
                for h in range(HPC):
                    col = qb * HPC + h
                    lhs = QT[h][:, qb * 128:(qb + 1) * 128]
                    exp_sb = mexp.tile([128, s], BF16, name="exp", tag="exp")
                    acc = macc.tile([128, NCH], F32, name="acc", tag="acc")
                    for ci, (c0, cl) in enumerate(chunks):
                        sc = spsum.tile([128, 1536], F32, name="sc", tag="sc")
                        for n in range(cl // 512):
                            nc.tensor.matmul(
                                sc[:, n * 512:(n + 1) * 512], lhs,
                                KT[h][:, c0 + n * 512:c0 + (n + 1) * 512])
                        nc.scalar.activation(
                            exp_sb[:, c0:c0 + cl], sc[:, :cl], AF.Exp,
                            scale=0.125, accum_out=acc[:, ci:ci + 1])
                    # softmax denominator -> 1/sum
                    ssum = macc.tile([128, 1], F32, name="ssum", tag="ssum")
                    nc.vector.tensor_reduce(
                        ssum[:], acc[:], axis=mybir.AxisListType.X, op=ALU.add)
                    nc.vector.reciprocal(inv_all[:, col:col + 1], ssum[:])
                    # normalized f32 attention out
                    attn_f = mattn.tile([128, s], F32, name="attnf", tag="attnf")
                    nc.vector.tensor_scalar(
                        attn_f[:], exp_sb[:], inv_all[:, col:col + 1], None,
                        ALU.mult)
                    nc.sync.dma_start(
                        attn_d[h, qb * 128:(qb + 1) * 128, :], attn_f[:])
                    # transpose exp -> [k, q] tiles, then attn @ V (unnormalized)
                    expT = mexpT.tile([128, s], BF16, name="expT", tag="expT")
                    for kt in range(NB):
                        nc.scalar.dma_start_transpose(
                            expT[:, kt * 128:(kt + 1) * 128],
                            exp_sb[:, kt * 128:(kt + 1) * 128])
                    ctx = cpsum.tile([HD, 128], F32, name="ctx", tag="ctx")
                    for kt in range(NB):
                        nc.tensor.matmul(
                            ctx[:], vh[kt][:, h * HD:(h + 1) * HD],
                            expT[:, kt * 128:(kt + 1) * 128],
                            start=(kt == 0), stop=(kt == NB - 1))
                    nc.vector.tensor_copy(
                        ctxT[h][:, qb * 128:(qb + 1) * 128], ctx[:])

        # ---------------- final projection ----------------
        with ExitStack() as fin:
            fpool = fin.enter_context(tc.tile_pool(name="fpool", bufs=3))
            fpsum = fin.enter_context(
                tc.tile_pool(name="fpsum", bufs=4, space=bass.MemorySpace.PSUM))
            for st in range(NB):
                os = []
                for h in range(HPC):
                    ps = fpsum.tile([128, HID], F32, name=f"fp{h}", tag=f"fp{h}")
                    nc.tensor.matmul(
                        ps[:], ctxT[h][:, st * 128:(st + 1) * 128], wfc_sb[h][:])
                    o = fpool.tile([128, HID], F32, name=f"o{h}", tag=f"o{h}")
                    nc.vector.tensor_scalar(
                        o[:], ps[:], inv_all[:, st * HPC + h:st * HPC + h + 1],
                        None, ALU.mult)
                    os.append(o)
                t2 = fpool.tile([128, HID], F32, name="t2", tag="t2")
                nc.gpsimd.tensor_tensor(t2[:], os[0][:], os[1][:], ALU.add)
                t3 = fpool.tile([128, HID], F32, name="t3", tag="t3")
                nc.gpsimd.tensor_tensor(t3[:], t2[:], bfc_bc[:], ALU.add)
                nc.sync.dma_start(pout_d[st * 128:(st + 1) * 128, :], t3[:])

    nc.compile()
    return nc


_nc_cache = {}


def _get_nc(s=S):
    if s not in _nc_cache:
        _nc_cache[s] = build(s)
    return _nc_cache[s]


def make_in_maps(q, k, v, masked, Wq, bq, Wk, bk, Wv, bv, Wfc, bfc, s=S):
    """Per-core input dicts. Core c -> batch c//4, heads (2*(c%4), 2*(c%4)+1)."""
    c32 = lambda a: np.ascontiguousarray(a, dtype=np.float32)
    in_maps = []
    gpb = NCORES // B  # core groups per batch
    for c in range(NCORES):
        b, g = divmod(c, gpb)
        col = slice(g * HPC * HD, (g + 1) * HPC * HD)
        madd = np.where(np.asarray(masked[b]).reshape(-1) == 0, -60000.0, 0.0)
        in_maps.append({
            "xq": c32(q[b]), "xk": c32(k[b]), "xv": c32(v[b]),
            "wq": c32(Wq[:, col]), "wk": c32(Wk[:, col]), "wv": c32(Wv[:, col]),
            "wfc": c32(Wfc[col, :]),
            "bq": c32(bq[col]).reshape(1, -1),
            "bk": c32(bk[col]).reshape(1, -1),
            "bv": c32(bv[col]).reshape(1, -1),
            "bfc": (c32(bfc) if g == 0 else np.zeros_like(c32(bfc))).reshape(1, -1),
            "madd": madd.astype(np.float16).reshape(1, -1),
        })
    return in_maps


def assemble(results, s=S):
    gpb = NCORES // B
    attention = np.empty((B, NH, s, s), np.float32)
    output = np.zeros((B, s, HID), np.float32)
    for c, r in enumerate(results):
        b, g = divmod(c, gpb)
        attention[b, g * HPC:(g + 1) * HPC] = r["attn"]
        output[b] += r["pout"]
    return output, attention


def kernel(q, k, v, masked, Wq, bq, Wk, bk, Wv, bv, Wfc, bfc):
    from concourse.bass_utils import run_bass_kernel_spmd
    nc = _get_nc(S)
    in_maps = make_in_maps(q, k, v, masked, Wq, bq, Wk, bk, Wv, bv, Wfc, bfc)
    res = run_bass_kernel_spmd(nc, in_maps, list(range(NCORES)))
    return assemble(res.results)
